# revision 7
# baseline (speedup 1.0000x reference)
"""Trainium2 Bass kernel for the DOC contrastive-loss module (epoch>=1 path).

Strategy (data-parallel over T, one frame per NeuronCore, 8 cores):

The reference computes, per frame, an L2-normalized pixel-feature Gram matrix
sim = f^T f / TEMP over N=H*W pixels, then
    pos    = exp(diag(sim))            (== exp(1/TEMP), since features are unit vectors)
    neg_n  = sum_{m in bg} exp(sim[n,m])
    l_n    = log(pos + neg_n + eps) - log(pos)
    frame_loss = mean_{n in fg} l_n,   loss = mean over valid frames.
Only the fg-rows x bg-cols block of the Gram matrix is ever needed, and pos is
a constant.  So the host (sharding step) partitions each frame's pixels into
fg/bg sets from the label maps, and each core computes:
    G = ffg_raw^T @ fbg_normalized     (PE, bf16, K=C=256)
    E = exp(G * rnorm_fg[row] / TEMP)  (ACT, fused row-sum accumulate -> neg)
    frame_loss from neg                (ACT/DVE epilogue, partition-sum via PE)
Zero-padding of the gathered fg/bg pixel sets is corrected exactly on device:
a padded bg column contributes exp(0)=1 per row (subtracted via the A constant)
and padded fg rows are masked out of the final sum.

Host-side work is limited to label selection / fg-bg index computation (the
sharding decision) and the trivial 8-way mean over frames.
"""

import math

import numpy as np

import concourse.bass as bass
import concourse.mybir as mybir
import concourse.tile as tile
from concourse import bacc
from concourse.bass_utils import run_bass_kernel_spmd

TEMP = 0.07
EPS = 1e-8
THRESH = 0.0
LOGPOS = 1.0 / TEMP  # log(pos) where pos = exp(1/TEMP)
POS = math.exp(LOGPOS)

AF = mybir.ActivationFunctionType
ALU = mybir.AluOpType
AX = mybir.AxisListType

_NC_CACHE: dict = {}
LAST_RESULTS = None  # BassKernelResults of the most recent run (for profiling)


def _build_nc(FG_TILES: int, BG_TILES: int, CB: int):
    """Build the SPMD single-core program: fg-rows x bg-cols masked softmax-denominator."""
    f32 = mybir.dt.float32
    bf16 = mybir.dt.bfloat16
    FGP = 128 * FG_TILES
    BGP = 128 * BG_TILES

    nc = bacc.Bacc("TRN2", target_bir_lowering=False, debug=False)

    ffg_d = nc.dram_tensor("ffg", [CB, 128, FGP], f32, kind="ExternalInput")
    fbg_d = nc.dram_tensor("fbg", [CB, 128, BGP], f32, kind="ExternalInput")
    rm_d = nc.dram_tensor("rowmask", [128, FG_TILES], f32, kind="ExternalInput")
    cst_d = nc.dram_tensor("consts", [128, 2], f32, kind="ExternalInput")
    out_d = nc.dram_tensor("out", [1, 1], f32, kind="ExternalOutput")

    # bg column chunks: big chunks (2 PSUM banks) + <=512 remainder
    chunks = []
    off = 0
    while off < BGP:
        w = min(1024, BGP - off)
        chunks.append((off, w))
        off += w
    NCH = len(chunks)

    with tile.TileContext(nc) as tc:
        with (
            tc.tile_pool(name="persist", bufs=1) as P,
            tc.tile_pool(name="scratch", bufs=3) as S,
            tc.tile_pool(name="prep_psum", bufs=2, space="PSUM") as PP,
            tc.tile_pool(name="mm_psum", bufs=3, space="PSUM") as MP,
        ):
            ones_bf = P.tile([128, 128], bf16)
            nc.vector.memset(ones_bf[:], 1.0)
            ones_f = P.tile([128, 1], f32)
            nc.vector.memset(ones_f[:], 1.0)
            bias_tiny = P.tile([128, 1], f32)
            nc.vector.memset(bias_tiny[:], 1e-30)
            bias_lnt = P.tile([128, 1], f32)
            nc.vector.memset(bias_lnt[:], float(math.log(1.0 / TEMP)))
            consts = P.tile([128, 2], f32)
            nc.sync.dma_start(consts[:], cst_d[:, :])
            rmask = P.tile([128, FG_TILES], f32)
            nc.sync.dma_start(rmask[:], rm_d[:, :])

            ffg32 = [P.tile([128, FGP], f32, name=f"ffg32_{c}") for c in range(CB)]
            fbg32 = [P.tile([128, BGP], f32, name=f"fbg32_{c}") for c in range(CB)]
            for c in range(CB):
                nc.sync.dma_start(fbg32[c][:], fbg_d[c, :, :])
            for c in range(CB):
                nc.sync.dma_start(ffg32[c][:], ffg_d[c, :, :])

            # ---- bg side: norms then normalize (bf16 out) ----
            f2bg = [P.tile([128, BGP], bf16, name=f"f2bg_{c}") for c in range(CB)]
            for c in range(CB):
                nc.vector.tensor_mul(f2bg[c][:], fbg32[c][:], fbg32[c][:])
            lnbg = P.tile([128, BGP], f32)
            for j0 in range(0, BGP, 512):
                w = min(512, BGP - j0)
                ps = PP.tile([128, w], f32, name="pprep", tag="pprep")
                for c in range(CB):
                    nc.tensor.matmul(
                        ps[:, :],
                        ones_bf[:, :],
                        f2bg[c][:, j0 : j0 + w],
                        start=(c == 0),
                        stop=(c == CB - 1),
                    )
                # ln(norm2); +1e-30 keeps padded (all-zero) columns finite
                nc.scalar.activation(
                    lnbg[:, j0 : j0 + w], ps[:, :], AF.Ln, bias=bias_tiny[:, :]
                )
            rnbg = P.tile([128, BGP], f32)
            # rnorm = exp(-0.5*ln(norm2)) = 1/sqrt(norm2); padded cols stay 0 after mult
            nc.scalar.activation(rnbg[:, :], lnbg[:, :], AF.Exp, scale=-0.5)
            fbgn = [P.tile([128, BGP], bf16, name=f"fbgn_{c}") for c in range(CB)]
            for c in range(CB):
                nc.vector.tensor_mul(fbgn[c][:], fbg32[c][:], rnbg[:, :])

            # ---- fg side: raw bf16 cast + per-row 1/(norm*TEMP) as ACT scale ----
            ffgbf = [P.tile([128, FGP], bf16, name=f"ffgbf_{c}") for c in range(CB)]
            for c in range(CB):
                nc.vector.tensor_copy(ffgbf[c][:], ffg32[c][:])
            f2fg = [P.tile([128, FGP], bf16, name=f"f2fg_{c}") for c in range(CB)]
            for c in range(CB):
                nc.vector.tensor_mul(f2fg[c][:], ffg32[c][:], ffg32[c][:])
            ps2 = PP.tile([128, FG_TILES], f32, name="ps2", tag="pprep")
            for i in range(FG_TILES):
                for c in range(CB):
                    nc.tensor.matmul(
                        ps2[:, i : i + 1],
                        f2fg[c][:, 128 * i : 128 * (i + 1)],
                        ones_bf[:, 0:1],
                        start=(c == 0),
                        stop=(c == CB - 1),
                    )
            lnfg = P.tile([128, FG_TILES], f32)
            nc.scalar.activation(lnfg[:, :], ps2[:, :], AF.Ln, bias=bias_tiny[:, :])
            scfg = P.tile([128, FG_TILES], f32)
            # scale_fg = exp(-0.5*ln(norm2) + ln(1/TEMP)) = 1/(norm*TEMP)
            nc.scalar.activation(
                scfg[:, :], lnfg[:, :], AF.Exp, scale=-0.5, bias=bias_lnt[:, :]
            )

            # ---- main loop: G tiles -> exp -> fused row-sum ----
            negacc = P.tile([128, FG_TILES * NCH], f32)
            for mi in range(FG_TILES):
                pts = []
                for off, w in chunks:
                    if w > 512:
                        pt = MP.tile([128, w], f32, name="g", tag="g")
                    else:
                        pt = PP.tile([128, w], f32, name="grem", tag="pprep")
                    pts.append(pt)
                for c in range(CB):
                    lhsT = ffgbf[c][:, 128 * mi : 128 * (mi + 1)]
                    for pt, (off, w) in zip(pts, chunks):
                        for s in range(0, w, 512):
                            ws = min(512, w - s)
                            nc.tensor.matmul(
                                pt[:, s : s + ws],
                                lhsT,
                                fbgn[c][:, off + s : off + s + ws],
                                start=(c == 0),
                                stop=(c == CB - 1),
                            )
                for j, (pt, (off, w)) in enumerate(zip(pts, chunks)):
                    es = S.tile([128, w], bf16, name="es", tag=f"es{j}")
                    nc.scalar.activation(
                        es[:, :],
                        pt[:, :],
                        AF.Exp,
                        scale=scfg[:, mi : mi + 1],
                        accum_out=negacc[:, mi * NCH + j : mi * NCH + j + 1],
                    )

            # ---- epilogue: frame_loss = inv_nfg * sum_fg (ln(neg_raw + A) - LOGPOS) ----
            negsum = P.tile([128, FG_TILES], f32)
            nc.vector.tensor_reduce(
                negsum[:, :],
                negacc[:, :].rearrange("p (m j) -> p m j", j=NCH),
                axis=AX.X,
                op=ALU.add,
            )
            plog = P.tile([128, FG_TILES], f32)
            # A = POS + EPS - n_bg_pad folds the padded-column correction into the log bias
            nc.scalar.activation(plog[:, :], negsum[:, :], AF.Ln, bias=consts[:, 0:1])
            masked = P.tile([128, FG_TILES], f32)
            nc.vector.scalar_tensor_tensor(
                masked[:, :], plog[:, :], -LOGPOS, rmask[:, :], op0=ALU.add, op1=ALU.mult
            )
            red = P.tile([128, 1], f32)
            nc.vector.tensor_reduce(red[:, :], masked[:, :], axis=AX.X, op=ALU.add)
            ps3 = PP.tile([1, 1], f32, name="ps3", tag="pprep")
            nc.tensor.matmul(ps3[:, :], red[:, :], ones_f[:, :])
            res = P.tile([1, 1], f32)
            nc.scalar.activation(res[:, :], ps3[:, :], AF.Copy, scale=consts[0:1, 1:2])
            nc.sync.dma_start(out_d[:, :], res[:, :])

    nc.compile()
    return nc


def _get_nc(FG_TILES: int, BG_TILES: int, CB: int):
    key = (FG_TILES, BG_TILES, CB)
    if key not in _NC_CACHE:
        _NC_CACHE[key] = _build_nc(FG_TILES, BG_TILES, CB)
    return _NC_CACHE[key]


def kernel(**inputs) -> np.ndarray:
    cur = np.asarray(inputs["current_preds"], dtype=np.float32)
    hist = np.asarray(inputs["history_preds"], dtype=np.float32)
    feats = np.asarray(inputs["features"], dtype=np.float32)

    T = cur.shape[0]
    C = feats.shape[1]
    N = int(np.prod(cur.shape[1:]))
    CB = C // 128

    # ---- labels (fp32 math mirroring the reference) ----
    cb = (cur > 0.5).astype(np.float32).reshape(T, -1)
    hb = (hist > 0.5).astype(np.float32).reshape(T, -1)
    e1 = (cb * hb).sum(axis=1, dtype=np.float32)
    e2 = cb.sum(axis=1, dtype=np.float32) + hb.sum(axis=1, dtype=np.float32)
    m1 = (np.float32(2.0) * e1 + np.float32(EPS)) / (e2 + np.float32(EPS))
    m2 = (e1 + np.float32(EPS)) / (e2 - e1 + np.float32(EPS))
    dev = np.float32(1.0) - (m1 + m2) / np.float32(2.0)
    use_curr = dev <= np.float32(THRESH)
    labels = np.where(use_curr[:, None, None, None], cur, hist).astype(np.float32)

    lbl = labels.reshape(T, N)
    fg = lbl > 0.5
    nfg = fg.sum(axis=1).astype(np.int64)
    nbg = N - nfg
    valid = (nfg > 0) & (nbg > 0)

    FG_TILES = max(1, int(-(-int(nfg.max()) // 128)))
    BG_TILES = max(1, int(-(-int(nbg.max()) // 128)))
    FGP, BGP = 128 * FG_TILES, 128 * BG_TILES

    in_maps = []
    for t in range(T):
        f = feats[t].reshape(C, N)
        m = fg[t]
        ffg = np.zeros((C, FGP), dtype=np.float32)
        ffg[:, : nfg[t]] = f[:, m]
        fbg = np.zeros((C, BGP), dtype=np.float32)
        fbg[:, : nbg[t]] = f[:, ~m]
        rowmask = (
            (np.arange(FGP).reshape(FG_TILES, 128).T < nfg[t]).astype(np.float32)
        )
        A = np.float32(POS + EPS - float(BGP - nbg[t]))
        inv_nfg = np.float32(1.0 / max(float(nfg[t]), 1.0))
        consts = np.tile(np.array([[A, inv_nfg]], dtype=np.float32), (128, 1))
        in_maps.append(
            {
                "ffg": np.ascontiguousarray(ffg.reshape(CB, 128, FGP)),
                "fbg": np.ascontiguousarray(fbg.reshape(CB, 128, BGP)),
                "rowmask": np.ascontiguousarray(rowmask),
                "consts": consts,
            }
        )

    nc = _get_nc(FG_TILES, BG_TILES, CB)
    res = run_bass_kernel_spmd(nc, in_maps, core_ids=list(range(T)))
    global LAST_RESULTS
    LAST_RESULTS = res

    fls = np.array([res.results[t]["out"][0, 0] for t in range(T)], dtype=np.float32)
    n_valid = int(valid.sum())
    if n_valid > 0:
        loss = np.float32((fls * valid.astype(np.float32)).sum() / max(n_valid, 1))
    else:
        loss = np.float32(0.0)
    return labels, np.asarray(loss, dtype=np.float32)


# revision 9
# speedup vs baseline: 1.1735x; 1.1735x over previous
"""Trainium2 Bass kernel for the DOC contrastive-loss module (epoch>=1 path).

Strategy (data-parallel over T, one frame per NeuronCore, 8 cores):

The reference computes, per frame, an L2-normalized pixel-feature Gram matrix
sim = f^T f / TEMP over N=H*W pixels, then
    pos    = exp(diag(sim))            (== exp(1/TEMP), since features are unit vectors)
    neg_n  = sum_{m in bg} exp(sim[n,m])
    l_n    = log(pos + neg_n + eps) - log(pos)
    frame_loss = mean_{n in fg} l_n,   loss = mean over valid frames.
Only the fg-rows x bg-cols block of the Gram matrix is ever needed, and pos is
a constant.  So the host (sharding step) partitions each frame's pixels into
fg/bg sets from the label maps, and each core computes:
    G = ffg_raw^T @ fbg_normalized     (PE, bf16, K=C=256)
    E = exp(G * rnorm_fg[row] / TEMP)  (ACT, fused row-sum accumulate -> neg)
    frame_loss from neg                (ACT/DVE epilogue, partition-sum via PE)
Zero-padding of the gathered fg/bg pixel sets is corrected exactly on device:
a padded bg column contributes exp(0)=1 per row (subtracted via the A constant)
and padded fg rows are masked out of the final sum.

Host-side work is limited to label selection / fg-bg index computation (the
sharding decision) and the trivial 8-way mean over frames.
"""

import functools
import math

import ml_dtypes
import numpy as np

import concourse.bass as bass
import concourse.mybir as mybir
import concourse.tile as tile
from concourse import bacc
from concourse.bass_utils import run_bass_kernel_spmd

TEMP = 0.07
EPS = 1e-8
THRESH = 0.0
LOGPOS = 1.0 / TEMP  # log(pos) where pos = exp(1/TEMP)
POS = math.exp(LOGPOS)

AF = mybir.ActivationFunctionType
ALU = mybir.AluOpType
AX = mybir.AxisListType

_NC_CACHE: dict = {}
LAST_RESULTS = None  # BassKernelResults of the most recent run (for profiling)


# ---------------------------------------------------------------------------
# Force every activation into the one table set that covers {Exp, Ln, Copy,
# Identity} so the program loads activation tables exactly once instead of
# ping-ponging between the exp and ln sets (~1.3us per load, serialized).
_ONE_SET = "natural_log_exp_and_others"
_orig_get_tables = None


def _patched_get_tables(arch):
    tabs = _orig_get_tables(arch)
    return {
        name: (funcs if name == _ONE_SET else frozenset())
        for name, funcs in tabs.items()
    }


def _install_act_table_patch():
    global _orig_get_tables
    if _orig_get_tables is not None:
        return
    from concourse import hw_specs

    _orig_get_tables = hw_specs.get_activation_tables
    patched = functools.cache(_patched_get_tables)
    hw_specs.get_activation_tables = patched
    bacc.get_activation_tables = patched


def _build_nc(FG_TILES: int, BG_TILES: int, CB: int):
    """SPMD single-core program: fg-rows x bg-cols masked softmax-denominator."""
    _install_act_table_patch()
    f32 = mybir.dt.float32
    bf16 = mybir.dt.bfloat16
    FGP = 128 * FG_TILES
    BGP = 128 * BG_TILES

    nc = bacc.Bacc("TRN2", target_bir_lowering=False, debug=False)

    ffg_d = nc.dram_tensor("ffg", [CB, 128, FGP], bf16, kind="ExternalInput")
    fbg_d = nc.dram_tensor("fbg", [CB, 128, BGP], bf16, kind="ExternalInput")
    rm_d = nc.dram_tensor("rowmask", [128, FG_TILES], f32, kind="ExternalInput")
    cst_d = nc.dram_tensor("consts", [128, 2], f32, kind="ExternalInput")
    out_d = nc.dram_tensor("out", [1, 1], f32, kind="ExternalOutput")

    # bg column chunks: 1024-wide (2 PSUM banks, fused exp+rowsum on ACT) plus a
    # <=512 remainder whose exp is batched into one big ACT pass at the end.
    big_chunks = []
    off = 0
    while off + 1024 <= BGP:
        big_chunks.append((off, 1024))
        off += 1024
    rem_off, rem_w = off, BGP - off  # 0 <= rem_w <= 512 when BGP % 1024 <= 512
    assert rem_w <= 512, "remainder chunk must fit one PSUM bank"
    NB = len(big_chunks)

    with tile.TileContext(nc) as tc:
        with (
            tc.tile_pool(name="persist", bufs=1) as P,
            tc.tile_pool(name="scratch", bufs=3) as S,
            tc.tile_pool(name="sp_psum", bufs=2, space="PSUM") as SP,
            tc.tile_pool(name="mm_psum", bufs=3, space="PSUM") as MP,
        ):
            ones_bf = P.tile([128, 128], bf16)
            nc.vector.memset(ones_bf[:], 1.0)
            ones_f = P.tile([128, 1], f32)
            nc.vector.memset(ones_f[:], 1.0)
            bias_tiny = P.tile([128, 1], f32)
            nc.vector.memset(bias_tiny[:], 1e-30)
            bias_lnt = P.tile([128, 1], f32)
            nc.vector.memset(bias_lnt[:], float(math.log(1.0 / TEMP)))
            consts = P.tile([128, 2], f32)
            nc.sync.dma_start(consts[:], cst_d[:, :])
            rmask = P.tile([128, FG_TILES], f32)
            nc.sync.dma_start(rmask[:], rm_d[:, :])

            # chunked input DMA so downstream prep can start early
            HB = BGP // 2
            HF = FGP // 2
            fbgb = [P.tile([128, BGP], bf16, name=f"fbgb_{c}") for c in range(CB)]
            ffgb = [P.tile([128, FGP], bf16, name=f"ffgb_{c}") for c in range(CB)]
            for c in range(CB):
                nc.sync.dma_start(fbgb[c][:, 0:HB], fbg_d[c, :, 0:HB])
                nc.sync.dma_start(ffgb[c][:, 0:HF], ffg_d[c, :, 0:HF])
                nc.sync.dma_start(fbgb[c][:, HB:BGP], fbg_d[c, :, HB:BGP])
                nc.sync.dma_start(ffgb[c][:, HF:FGP], ffg_d[c, :, HF:FGP])

            # ---- bg norms: f2 -> ones-matmul -> ln -> exp(-0.5 ln) -> normalize ----
            all_chunks = big_chunks + ([(rem_off, rem_w)] if rem_w else [])
            f2bg = [P.tile([128, BGP], bf16, name=f"f2bg_{c}") for c in range(CB)]
            for c in range(CB):
                for j0 in (0, HB):
                    nc.vector.tensor_mul(
                        f2bg[c][:, j0 : j0 + HB],
                        fbgb[c][:, j0 : j0 + HB],
                        fbgb[c][:, j0 : j0 + HB],
                    )
            lnbg = P.tile([128, BGP], f32)
            for off, w in all_chunks:
                ps = (
                    MP.tile([128, w], f32, name="g", tag="g")
                    if w > 512
                    else SP.tile([128, w], f32, name="sp", tag="sp")
                )
                for c in range(CB):
                    for s in range(0, w, 512):
                        ws = min(512, w - s)
                        nc.tensor.matmul(
                            ps[:, s : s + ws],
                            ones_bf[:, :],
                            f2bg[c][:, off + s : off + s + ws],
                            start=(c == 0),
                            stop=(c == CB - 1),
                        )
                nc.scalar.activation(
                    lnbg[:, off : off + w], ps[:, :], AF.Ln, bias=bias_tiny[:, :]
                )
            rnbg = P.tile([128, BGP], bf16)
            for off, w in all_chunks:
                # rnorm = exp(-0.5*ln(norm2)) = 1/sqrt(norm2); padded cols stay 0
                nc.scalar.activation(
                    rnbg[:, off : off + w], lnbg[:, off : off + w], AF.Exp, scale=-0.5
                )
            fbgn = [P.tile([128, BGP], bf16, name=f"fbgn_{c}") for c in range(CB)]
            for c in range(CB):
                for j0 in (0, HB):
                    nc.vector.tensor_mul(
                        fbgn[c][:, j0 : j0 + HB],
                        fbgb[c][:, j0 : j0 + HB],
                        rnbg[:, j0 : j0 + HB],
                    )

            # ---- fg norms -> per-row ACT scale 1/(norm*TEMP) ----
            f2fg = [P.tile([128, FGP], bf16, name=f"f2fg_{c}") for c in range(CB)]
            for c in range(CB):
                for j0 in (0, HF):
                    nc.vector.tensor_mul(
                        f2fg[c][:, j0 : j0 + HF],
                        ffgb[c][:, j0 : j0 + HF],
                        ffgb[c][:, j0 : j0 + HF],
                    )
            ps2 = SP.tile([128, FG_TILES], f32, name="ps2", tag="sp")
            for i in range(FG_TILES):
                for c in range(CB):
                    nc.tensor.matmul(
                        ps2[:, i : i + 1],
                        f2fg[c][:, 128 * i : 128 * (i + 1)],
                        ones_bf[:, 0:1],
                        start=(c == 0),
                        stop=(c == CB - 1),
                    )
            lnfg = P.tile([128, FG_TILES], f32)
            nc.scalar.activation(lnfg[:, :], ps2[:, :], AF.Ln, bias=bias_tiny[:, :])
            scfg = P.tile([128, FG_TILES], f32)
            # scale_fg = exp(-0.5*ln(norm2) + ln(1/TEMP)) = 1/(norm*TEMP)
            nc.scalar.activation(
                scfg[:, :], lnfg[:, :], AF.Exp, scale=-0.5, bias=bias_lnt[:, :]
            )

            # ---- main loop: G tiles -> exp -> fused row-sum on ACT ----
            negacc = P.tile([128, FG_TILES * NB], f32)
            stage_rem = (
                P.tile([128, FG_TILES * rem_w], f32, name="stage_rem")
                if rem_w
                else None
            )
            for mi in range(FG_TILES):
                gts = [MP.tile([128, w], f32, name="g", tag="g") for _, w in big_chunks]
                prem = (
                    SP.tile([128, rem_w], f32, name="sp", tag="sp") if rem_w else None
                )
                for c in range(CB):
                    lhsT = ffgb[c][:, 128 * mi : 128 * (mi + 1)]
                    for gt, (off, w) in zip(gts, big_chunks):
                        for s in range(0, w, 512):
                            nc.tensor.matmul(
                                gt[:, s : s + 512],
                                lhsT,
                                fbgn[c][:, off + s : off + s + 512],
                                start=(c == 0),
                                stop=(c == CB - 1),
                            )
                    if rem_w:
                        nc.tensor.matmul(
                            prem[:, :],
                            lhsT,
                            fbgn[c][:, rem_off : rem_off + rem_w],
                            start=(c == 0),
                            stop=(c == CB - 1),
                        )
                for j, gt in enumerate(gts):
                    es = S.tile([128, 1024], bf16, name="es", tag=f"es{j}")
                    nc.scalar.activation(
                        es[:, :],
                        gt[:, :],
                        AF.Exp,
                        scale=scfg[:, mi : mi + 1],
                        accum_out=negacc[:, mi * NB + j : mi * NB + j + 1],
                    )
                if rem_w:
                    # defer the narrow remainder: pre-scale rows now, exp later
                    nc.vector.tensor_scalar_mul(
                        stage_rem[:, mi * rem_w : (mi + 1) * rem_w],
                        prem[:, :],
                        scfg[:, mi : mi + 1],
                    )

            # ---- epilogue ----
            negsum = P.tile([128, FG_TILES], f32)
            nc.vector.tensor_reduce(
                negsum[:, :],
                negacc[:, :].rearrange("p (m j) -> p m j", j=NB),
                axis=AX.X,
                op=ALU.add,
            )
            if rem_w:
                erem = S.tile([128, FG_TILES * rem_w], bf16, name="erem", tag="erem")
                nc.scalar.activation(erem[:, :], stage_rem[:, :], AF.Exp)
                remsum = P.tile([128, FG_TILES], f32)
                nc.vector.tensor_reduce(
                    remsum[:, :],
                    erem[:, :].rearrange("p (m j) -> p m j", j=rem_w),
                    axis=AX.X,
                    op=ALU.add,
                )
                nc.vector.tensor_add(negsum[:, :], negsum[:, :], remsum[:, :])
            plog = P.tile([128, FG_TILES], f32)
            # A = POS + EPS - n_bg_pad folds the padded-column correction into the bias
            nc.scalar.activation(plog[:, :], negsum[:, :], AF.Ln, bias=consts[:, 0:1])
            masked = P.tile([128, FG_TILES], f32)
            nc.vector.scalar_tensor_tensor(
                masked[:, :], plog[:, :], -LOGPOS, rmask[:, :], op0=ALU.add, op1=ALU.mult
            )
            red = P.tile([128, 1], f32)
            nc.vector.tensor_reduce(red[:, :], masked[:, :], axis=AX.X, op=ALU.add)
            ps3 = SP.tile([1, 1], f32, name="sp", tag="sp")
            nc.tensor.matmul(ps3[:, :], red[:, :], ones_f[:, :])
            res = P.tile([1, 1], f32)
            nc.scalar.activation(res[:, :], ps3[:, :], AF.Copy, scale=consts[0:1, 1:2])
            nc.sync.dma_start(out_d[:, :], res[:, :])

    nc.compile()
    return nc


def _get_nc(FG_TILES: int, BG_TILES: int, CB: int):
    key = (FG_TILES, BG_TILES, CB)
    if key not in _NC_CACHE:
        _NC_CACHE[key] = _build_nc(FG_TILES, BG_TILES, CB)
    return _NC_CACHE[key]


def kernel(**inputs) -> np.ndarray:
    cur = np.asarray(inputs["current_preds"], dtype=np.float32)
    hist = np.asarray(inputs["history_preds"], dtype=np.float32)
    feats = np.asarray(inputs["features"], dtype=np.float32)

    T = cur.shape[0]
    C = feats.shape[1]
    N = int(np.prod(cur.shape[1:]))
    CB = C // 128

    # ---- labels (fp32 math mirroring the reference) ----
    cb = (cur > 0.5).astype(np.float32).reshape(T, -1)
    hb = (hist > 0.5).astype(np.float32).reshape(T, -1)
    e1 = (cb * hb).sum(axis=1, dtype=np.float32)
    e2 = cb.sum(axis=1, dtype=np.float32) + hb.sum(axis=1, dtype=np.float32)
    m1 = (np.float32(2.0) * e1 + np.float32(EPS)) / (e2 + np.float32(EPS))
    m2 = (e1 + np.float32(EPS)) / (e2 - e1 + np.float32(EPS))
    dev = np.float32(1.0) - (m1 + m2) / np.float32(2.0)
    use_curr = dev <= np.float32(THRESH)
    labels = np.where(use_curr[:, None, None, None], cur, hist).astype(np.float32)

    lbl = labels.reshape(T, N)
    fg = lbl > 0.5
    nfg = fg.sum(axis=1).astype(np.int64)
    nbg = N - nfg
    valid = (nfg > 0) & (nbg > 0)

    FG_TILES = max(1, int(-(-int(nfg.max()) // 128)))
    BG_TILES = max(1, int(-(-int(nbg.max()) // 128)))
    # keep the BGP remainder chunk within one PSUM bank (<=512 columns)
    while (128 * BG_TILES) % 1024 > 512:
        BG_TILES += 1
    FGP, BGP = 128 * FG_TILES, 128 * BG_TILES

    in_maps = []
    for t in range(T):
        f = feats[t].reshape(C, N)
        m = fg[t]
        ffg = np.zeros((C, FGP), dtype=ml_dtypes.bfloat16)
        ffg[:, : nfg[t]] = f[:, m].astype(ml_dtypes.bfloat16)
        fbg = np.zeros((C, BGP), dtype=ml_dtypes.bfloat16)
        fbg[:, : nbg[t]] = f[:, ~m].astype(ml_dtypes.bfloat16)
        rowmask = (
            (np.arange(FGP).reshape(FG_TILES, 128).T < nfg[t]).astype(np.float32)
        )
        A = np.float32(POS + EPS - float(BGP - nbg[t]))
        inv_nfg = np.float32(1.0 / max(float(nfg[t]), 1.0))
        consts = np.tile(np.array([[A, inv_nfg]], dtype=np.float32), (128, 1))
        in_maps.append(
            {
                "ffg": np.ascontiguousarray(ffg.reshape(CB, 128, FGP)),
                "fbg": np.ascontiguousarray(fbg.reshape(CB, 128, BGP)),
                "rowmask": np.ascontiguousarray(rowmask),
                "consts": consts,
            }
        )

    nc = _get_nc(FG_TILES, BG_TILES, CB)
    res = run_bass_kernel_spmd(nc, in_maps, core_ids=list(range(T)))
    global LAST_RESULTS
    LAST_RESULTS = res

    fls = np.array([res.results[t]["out"][0, 0] for t in range(T)], dtype=np.float32)
    n_valid = int(valid.sum())
    if n_valid > 0:
        loss = np.float32((fls * valid.astype(np.float32)).sum() / max(n_valid, 1))
    else:
        loss = np.float32(0.0)
    return labels, np.asarray(loss, dtype=np.float32)


# revision 10
# speedup vs baseline: 1.1994x; 1.0221x over previous
"""Trainium2 Bass kernel for the DOC contrastive-loss module (epoch>=1 path).

Strategy (data-parallel over T, one frame per NeuronCore, 8 cores):

The reference computes, per frame, an L2-normalized pixel-feature Gram matrix
sim = f^T f / TEMP over N=H*W pixels, then
    pos    = exp(diag(sim))            (== exp(1/TEMP), since features are unit vectors)
    neg_n  = sum_{m in bg} exp(sim[n,m])
    l_n    = log(pos + neg_n + eps) - log(pos)
    frame_loss = mean_{n in fg} l_n,   loss = mean over valid frames.
Only the fg-rows x bg-cols block of the Gram matrix is ever needed, and pos is
a constant.  So the host (sharding step) partitions each frame's pixels into
fg/bg sets from the label maps, and each core computes:
    G = ffg_raw^T @ fbg_normalized     (PE, bf16, K=C=256)
    E = exp(G * rnorm_fg[row] / TEMP)  (ACT, fused row-sum accumulate -> neg)
    frame_loss from neg                (ACT/DVE epilogue, partition-sum via PE)
Zero-padding of the gathered fg/bg pixel sets is corrected exactly on device:
a padded bg column contributes exp(0)=1 per row (subtracted via the A constant)
and padded fg rows are masked out of the final sum.

Host-side work is limited to label selection / fg-bg index computation (the
sharding decision) and the trivial 8-way mean over frames.
"""

import functools
import math

import ml_dtypes
import numpy as np

import concourse.bass as bass
import concourse.mybir as mybir
import concourse.tile as tile
from concourse import bacc
from concourse.bass_utils import run_bass_kernel_spmd

TEMP = 0.07
EPS = 1e-8
THRESH = 0.0
LOGPOS = 1.0 / TEMP  # log(pos) where pos = exp(1/TEMP)
POS = math.exp(LOGPOS)

AF = mybir.ActivationFunctionType
ALU = mybir.AluOpType
AX = mybir.AxisListType

_NC_CACHE: dict = {}
LAST_RESULTS = None  # BassKernelResults of the most recent run (for profiling)


# ---------------------------------------------------------------------------
# Force every activation into the one table set that covers {Exp, Ln, Copy,
# Identity} so the program loads activation tables exactly once instead of
# ping-ponging between the exp and ln sets (~1.3us per load, serialized).
_ONE_SET = "natural_log_exp_and_others"
_orig_get_tables = None


def _patched_get_tables(arch):
    tabs = _orig_get_tables(arch)
    return {
        name: (funcs if name == _ONE_SET else frozenset())
        for name, funcs in tabs.items()
    }


def _install_act_table_patch():
    global _orig_get_tables
    if _orig_get_tables is not None:
        return
    from concourse import hw_specs

    _orig_get_tables = hw_specs.get_activation_tables
    patched = functools.cache(_patched_get_tables)
    hw_specs.get_activation_tables = patched
    bacc.get_activation_tables = patched


def _build_nc(FG_TILES: int, BG_TILES: int, CB: int):
    """SPMD single-core program: fg-rows x bg-cols masked softmax-denominator."""
    _install_act_table_patch()
    f32 = mybir.dt.float32
    bf16 = mybir.dt.bfloat16
    FGP = 128 * FG_TILES
    BGP = 128 * BG_TILES

    nc = bacc.Bacc("TRN2", target_bir_lowering=False, debug=False)

    ffg_d = nc.dram_tensor("ffg", [CB, 128, FGP], bf16, kind="ExternalInput")
    fbg_d = nc.dram_tensor("fbg", [CB, 128, BGP], bf16, kind="ExternalInput")
    rm_d = nc.dram_tensor("rowmask", [128, FG_TILES], f32, kind="ExternalInput")
    cst_d = nc.dram_tensor("consts", [128, 2], f32, kind="ExternalInput")
    out_d = nc.dram_tensor("out", [1, 1], f32, kind="ExternalOutput")

    # bg column chunks: 1024-wide (2 PSUM banks, fused exp+rowsum on ACT) plus a
    # <=512 remainder whose exp is batched into one big ACT pass at the end.
    big_chunks = []
    off = 0
    while off + 1024 <= BGP:
        big_chunks.append((off, 1024))
        off += 1024
    rem_off, rem_w = off, BGP - off  # 0 <= rem_w <= 512 when BGP % 1024 <= 512
    assert rem_w <= 512, "remainder chunk must fit one PSUM bank"
    NB = len(big_chunks)

    with tile.TileContext(nc) as tc:
        with (
            tc.tile_pool(name="persist", bufs=1) as P,
            tc.tile_pool(name="scratch", bufs=3) as S,
            tc.tile_pool(name="sp_psum", bufs=2, space="PSUM") as SP,
            tc.tile_pool(name="mm_psum", bufs=3, space="PSUM") as MP,
        ):
            ones_bf = P.tile([128, 128], bf16)
            nc.vector.memset(ones_bf[:], 1.0)
            ones_f = P.tile([128, 1], f32)
            nc.vector.memset(ones_f[:], 1.0)
            bias_tiny = P.tile([128, 1], f32)
            nc.vector.memset(bias_tiny[:], 1e-30)
            bias_lnt = P.tile([128, 1], f32)
            nc.vector.memset(bias_lnt[:], float(math.log(1.0 / TEMP)))
            consts = P.tile([128, 2], f32)
            nc.sync.dma_start(consts[:], cst_d[:, :])
            rmask = P.tile([128, FG_TILES], f32)
            nc.sync.dma_start(rmask[:], rm_d[:, :])

            # ---- chunked input DMA, first bg chunk prioritized so the main loop
            # can start as soon as one bg chunk + the fg scales are ready ----
            all_chunks = big_chunks + ([(rem_off, rem_w)] if rem_w else [])
            HF = FGP // 2
            fbgb = [P.tile([128, BGP], bf16, name=f"fbgb_{c}") for c in range(CB)]
            ffgb = [P.tile([128, FGP], bf16, name=f"ffgb_{c}") for c in range(CB)]
            c0off, c0w = all_chunks[0]
            for c in range(CB):
                nc.sync.dma_start(fbgb[c][:, c0off : c0off + c0w], fbg_d[c, :, c0off : c0off + c0w])
            for c in range(CB):
                nc.sync.dma_start(ffgb[c][:, 0:HF], ffg_d[c, :, 0:HF])
                nc.sync.dma_start(ffgb[c][:, HF:FGP], ffg_d[c, :, HF:FGP])
            for off, w in all_chunks[1:]:
                for c in range(CB):
                    nc.sync.dma_start(fbgb[c][:, off : off + w], fbg_d[c, :, off : off + w])

            # ---- bg pipeline, chunk-major: f2 -> ones-matmul -> ln -> exp -> normalize
            f2bg = [P.tile([128, BGP], bf16, name=f"f2bg_{c}") for c in range(CB)]
            lnbg = P.tile([128, BGP], f32)
            rnbg = P.tile([128, BGP], bf16)
            fbgn = [P.tile([128, BGP], bf16, name=f"fbgn_{c}") for c in range(CB)]

            def bg_norm_chunk(off, w):
                for c in range(CB):
                    nc.vector.tensor_mul(
                        f2bg[c][:, off : off + w],
                        fbgb[c][:, off : off + w],
                        fbgb[c][:, off : off + w],
                    )
                ps = (
                    MP.tile([128, w], f32, name="g", tag="g")
                    if w > 512
                    else SP.tile([128, w], f32, name="sp", tag="sp")
                )
                for c in range(CB):
                    for s in range(0, w, 512):
                        ws = min(512, w - s)
                        nc.tensor.matmul(
                            ps[:, s : s + ws],
                            ones_bf[:, :],
                            f2bg[c][:, off + s : off + s + ws],
                            start=(c == 0),
                            stop=(c == CB - 1),
                        )
                # ln(norm2); +1e-30 keeps padded (all-zero) columns finite
                nc.scalar.activation(
                    lnbg[:, off : off + w], ps[:, :], AF.Ln, bias=bias_tiny[:, :]
                )
                # rnorm = exp(-0.5*ln(norm2)) = 1/sqrt(norm2); padded cols stay 0
                nc.scalar.activation(
                    rnbg[:, off : off + w], lnbg[:, off : off + w], AF.Exp, scale=-0.5
                )
                for c in range(CB):
                    nc.vector.tensor_mul(
                        fbgn[c][:, off : off + w],
                        fbgb[c][:, off : off + w],
                        rnbg[:, off : off + w],
                    )

            bg_norm_chunk(*all_chunks[0])

            # ---- fg norms -> per-row ACT scale 1/(norm*TEMP) ----
            f2fg = [P.tile([128, FGP], bf16, name=f"f2fg_{c}") for c in range(CB)]
            for c in range(CB):
                for j0 in (0, HF):
                    nc.vector.tensor_mul(
                        f2fg[c][:, j0 : j0 + HF],
                        ffgb[c][:, j0 : j0 + HF],
                        ffgb[c][:, j0 : j0 + HF],
                    )
            ps2 = SP.tile([128, FG_TILES], f32, name="ps2", tag="sp")
            for i in range(FG_TILES):
                for c in range(CB):
                    nc.tensor.matmul(
                        ps2[:, i : i + 1],
                        f2fg[c][:, 128 * i : 128 * (i + 1)],
                        ones_bf[:, 0:1],
                        start=(c == 0),
                        stop=(c == CB - 1),
                    )
            lnfg = P.tile([128, FG_TILES], f32)
            nc.scalar.activation(lnfg[:, :], ps2[:, :], AF.Ln, bias=bias_tiny[:, :])
            scfg = P.tile([128, FG_TILES], f32)
            # scale_fg = exp(-0.5*ln(norm2) + ln(1/TEMP)) = 1/(norm*TEMP)
            nc.scalar.activation(
                scfg[:, :], lnfg[:, :], AF.Exp, scale=-0.5, bias=bias_lnt[:, :]
            )

            for off, w in all_chunks[1:]:
                bg_norm_chunk(off, w)

            # ---- main loop: G tiles -> exp -> fused row-sum on ACT ----
            negacc = P.tile([128, FG_TILES * NB], f32)
            stage_rem = (
                P.tile([128, FG_TILES * rem_w], f32, name="stage_rem")
                if rem_w
                else None
            )
            for mi in range(FG_TILES):
                gts = [MP.tile([128, w], f32, name="g", tag="g") for _, w in big_chunks]
                prem = (
                    SP.tile([128, rem_w], f32, name="sp", tag="sp") if rem_w else None
                )
                for c in range(CB):
                    lhsT = ffgb[c][:, 128 * mi : 128 * (mi + 1)]
                    for gt, (off, w) in zip(gts, big_chunks):
                        for s in range(0, w, 512):
                            nc.tensor.matmul(
                                gt[:, s : s + 512],
                                lhsT,
                                fbgn[c][:, off + s : off + s + 512],
                                start=(c == 0),
                                stop=(c == CB - 1),
                            )
                    if rem_w:
                        nc.tensor.matmul(
                            prem[:, :],
                            lhsT,
                            fbgn[c][:, rem_off : rem_off + rem_w],
                            start=(c == 0),
                            stop=(c == CB - 1),
                        )
                for j, gt in enumerate(gts):
                    es = S.tile([128, 1024], bf16, name="es", tag=f"es{j}")
                    nc.scalar.activation(
                        es[:, :],
                        gt[:, :],
                        AF.Exp,
                        scale=scfg[:, mi : mi + 1],
                        accum_out=negacc[:, mi * NB + j : mi * NB + j + 1],
                    )
                if rem_w:
                    # defer the narrow remainder: pre-scale rows now, exp later
                    nc.vector.tensor_scalar_mul(
                        stage_rem[:, mi * rem_w : (mi + 1) * rem_w],
                        prem[:, :],
                        scfg[:, mi : mi + 1],
                    )

            # ---- epilogue ----
            negsum = P.tile([128, FG_TILES], f32)
            nc.vector.tensor_reduce(
                negsum[:, :],
                negacc[:, :].rearrange("p (m j) -> p m j", j=NB),
                axis=AX.X,
                op=ALU.add,
            )
            if rem_w:
                erem = S.tile([128, FG_TILES * rem_w], bf16, name="erem", tag="erem")
                nc.scalar.activation(erem[:, :], stage_rem[:, :], AF.Exp)
                remsum = P.tile([128, FG_TILES], f32)
                nc.vector.tensor_reduce(
                    remsum[:, :],
                    erem[:, :].rearrange("p (m j) -> p m j", j=rem_w),
                    axis=AX.X,
                    op=ALU.add,
                )
                nc.vector.tensor_add(negsum[:, :], negsum[:, :], remsum[:, :])
            plog = P.tile([128, FG_TILES], f32)
            # A = POS + EPS - n_bg_pad folds the padded-column correction into the bias
            nc.scalar.activation(plog[:, :], negsum[:, :], AF.Ln, bias=consts[:, 0:1])
            masked = P.tile([128, FG_TILES], f32)
            nc.vector.scalar_tensor_tensor(
                masked[:, :], plog[:, :], -LOGPOS, rmask[:, :], op0=ALU.add, op1=ALU.mult
            )
            red = P.tile([128, 1], f32)
            nc.vector.tensor_reduce(red[:, :], masked[:, :], axis=AX.X, op=ALU.add)
            ps3 = SP.tile([1, 1], f32, name="sp", tag="sp")
            nc.tensor.matmul(ps3[:, :], red[:, :], ones_f[:, :])
            res = P.tile([1, 1], f32)
            nc.scalar.activation(res[:, :], ps3[:, :], AF.Copy, scale=consts[0:1, 1:2])
            nc.sync.dma_start(out_d[:, :], res[:, :])

    nc.compile()
    return nc


def _get_nc(FG_TILES: int, BG_TILES: int, CB: int):
    key = (FG_TILES, BG_TILES, CB)
    if key not in _NC_CACHE:
        _NC_CACHE[key] = _build_nc(FG_TILES, BG_TILES, CB)
    return _NC_CACHE[key]


def kernel(**inputs) -> np.ndarray:
    cur = np.asarray(inputs["current_preds"], dtype=np.float32)
    hist = np.asarray(inputs["history_preds"], dtype=np.float32)
    feats = np.asarray(inputs["features"], dtype=np.float32)

    T = cur.shape[0]
    C = feats.shape[1]
    N = int(np.prod(cur.shape[1:]))
    CB = C // 128

    # ---- labels (fp32 math mirroring the reference) ----
    cb = (cur > 0.5).astype(np.float32).reshape(T, -1)
    hb = (hist > 0.5).astype(np.float32).reshape(T, -1)
    e1 = (cb * hb).sum(axis=1, dtype=np.float32)
    e2 = cb.sum(axis=1, dtype=np.float32) + hb.sum(axis=1, dtype=np.float32)
    m1 = (np.float32(2.0) * e1 + np.float32(EPS)) / (e2 + np.float32(EPS))
    m2 = (e1 + np.float32(EPS)) / (e2 - e1 + np.float32(EPS))
    dev = np.float32(1.0) - (m1 + m2) / np.float32(2.0)
    use_curr = dev <= np.float32(THRESH)
    labels = np.where(use_curr[:, None, None, None], cur, hist).astype(np.float32)

    lbl = labels.reshape(T, N)
    fg = lbl > 0.5
    nfg = fg.sum(axis=1).astype(np.int64)
    nbg = N - nfg
    valid = (nfg > 0) & (nbg > 0)

    FG_TILES = max(1, int(-(-int(nfg.max()) // 128)))
    BG_TILES = max(1, int(-(-int(nbg.max()) // 128)))
    # keep the BGP remainder chunk within one PSUM bank (<=512 columns)
    while (128 * BG_TILES) % 1024 > 512:
        BG_TILES += 1
    FGP, BGP = 128 * FG_TILES, 128 * BG_TILES

    in_maps = []
    for t in range(T):
        f = feats[t].reshape(C, N)
        m = fg[t]
        ffg = np.zeros((C, FGP), dtype=ml_dtypes.bfloat16)
        ffg[:, : nfg[t]] = f[:, m].astype(ml_dtypes.bfloat16)
        fbg = np.zeros((C, BGP), dtype=ml_dtypes.bfloat16)
        fbg[:, : nbg[t]] = f[:, ~m].astype(ml_dtypes.bfloat16)
        rowmask = (
            (np.arange(FGP).reshape(FG_TILES, 128).T < nfg[t]).astype(np.float32)
        )
        A = np.float32(POS + EPS - float(BGP - nbg[t]))
        inv_nfg = np.float32(1.0 / max(float(nfg[t]), 1.0))
        consts = np.tile(np.array([[A, inv_nfg]], dtype=np.float32), (128, 1))
        in_maps.append(
            {
                "ffg": np.ascontiguousarray(ffg.reshape(CB, 128, FGP)),
                "fbg": np.ascontiguousarray(fbg.reshape(CB, 128, BGP)),
                "rowmask": np.ascontiguousarray(rowmask),
                "consts": consts,
            }
        )

    nc = _get_nc(FG_TILES, BG_TILES, CB)
    res = run_bass_kernel_spmd(nc, in_maps, core_ids=list(range(T)))
    global LAST_RESULTS
    LAST_RESULTS = res

    fls = np.array([res.results[t]["out"][0, 0] for t in range(T)], dtype=np.float32)
    n_valid = int(valid.sum())
    if n_valid > 0:
        loss = np.float32((fls * valid.astype(np.float32)).sum() / max(n_valid, 1))
    else:
        loss = np.float32(0.0)
    return labels, np.asarray(loss, dtype=np.float32)
